# revision 12
# baseline (speedup 1.0000x reference)
"""Trainium2 Bass kernel for y = x @ W.T + b  (x: [16384,1024] f32,
W: [1024,1024] f32, b: [1024] f32) on 8 NeuronCores.

Data-parallel: x is split along batch into 8 shards of 2048 rows;
W and b are replicated. Each core computes its y shard with bf16
matmuls accumulating in fp32 PSUM; bias is fused into the PSUM->SBUF
eviction on the Scalar engine. Host-side we pre-transpose x (and W) to
put the contraction dim on SBUF partitions, so no on-chip transposes
are needed, and group DRAM layouts so every DMA is 128 long contiguous
runs (DMA issue time scales with descriptor rows).

Schedule per core (bq = one of 4 batch chunks of 512 rows):
- Dummy matmuls warm the PE clock gate (1.2 -> 2.4 GHz takes ~3.4 us
  of busy) while the first input DMAs are in flight.
- bq0 runs contraction-outer across all 8 PSUM banks, consuming one
  (w[ko], x[ko]) chunk pair per 1.7 us — matched to the observed DMA
  rate so the matmul stream never starves while inputs land.
- bq1..3 run output-tile-outer (one PSUM bank at a time), evicting
  each bank through the Scalar engine while later tiles compute.
- Outputs stream out per batch chunk; the last chunk is split into
  four stores so the final DMA doesn't lengthen the kernel tail.
"""

import sys

if "/opt/trn_rl_repo" not in sys.path:
    sys.path.insert(0, "/opt/trn_rl_repo")

import ml_dtypes
import numpy as np

BATCH = 16384
IN_F = 1024
OUT_F = 1024
NCORES = 8
P = 128
KO = IN_F // P  # 8 contraction tiles
MO = OUT_F // P  # 8 output-feature tiles
BS = BATCH // NCORES  # 2048 rows per core
FD = 512  # matmul moving free dim (one PSUM bank of fp32)
NB = BS // FD  # 4 batch chunks per core

_cache = {}
LAST_RESULT = None


def _build():
    import concourse.mybir as mybir
    import concourse.tile as tile
    from concourse import bacc

    nc = bacc.Bacc(None, target_bir_lowering=False)
    # xT4[p, bq, ko, fd] = x[bq*FD + fd, ko*P + p]
    xT = nc.declare_dram_parameter(
        "xT", [P, NB, KO, FD], mybir.dt.bfloat16, isOutput=False
    )
    # w3[p, ko, mo, c] = W[mo*P + c, ko*P + p]  (ko-major: bq0 consumes
    # weights one ko chunk at a time)
    w3 = nc.declare_dram_parameter(
        "w3", [P, KO, MO, P], mybir.dt.bfloat16, isOutput=False
    )
    bias = nc.declare_dram_parameter("bias", [P, MO], mybir.dt.float32, isOutput=False)
    # out4[p, bq, mo, fd] = y[bq*FD + fd, mo*P + p]
    out = nc.declare_dram_parameter(
        "out", [P, NB, MO, FD], mybir.dt.float32, isOutput=True
    )

    with tile.TileContext(nc) as tc:
        with (
            tc.tile_pool(name="const", bufs=1) as cpool,
            tc.tile_pool(name="outp", bufs=3) as opool,
            tc.tile_pool(name="psum", bufs=8, space="PSUM") as ppool,
        ):
            x_sb = cpool.tile([P, NB, KO, FD], mybir.dt.bfloat16)
            w_sb = cpool.tile([P, KO, MO, P], mybir.dt.bfloat16)
            b_sb = cpool.tile([P, MO], mybir.dt.float32)
            # PE HAM warm-up (shares the psum pool slots with the real
            # accumulation tiles; it finishes before they are needed).
            wu_sb = cpool.tile([P, 256], mybir.dt.bfloat16)
            nc.any.memset(wu_sb[:], 0.0)
            wu_ps = ppool.tile([P, FD], mybir.dt.float32, tag="ps")
            for _ in range(20):
                nc.tensor.matmul(
                    wu_ps[:, :256], wu_sb[:, :P], wu_sb[:], start=True, stop=True
                )
            # DMA issue order matches consumption order: (w, x) chunk pairs
            # for bq0 one ko at a time, then the remaining batch chunks as
            # one large contiguous DMA each.
            for ko in range(KO):
                nc.sync.dma_start(w_sb[:, ko], w3[:, ko])
                nc.sync.dma_start(x_sb[:, 0, ko], xT[:, 0, ko])
            nc.sync.dma_start(b_sb[:], bias[:])
            for bq in range(1, NB):
                nc.sync.dma_start(x_sb[:, bq], xT[:, bq])

            # bq0: contraction-outer over all 8 PSUM banks.
            ps0 = [
                ppool.tile([P, FD], mybir.dt.float32, tag="ps", name=f"ps0_{mo}")
                for mo in range(MO)
            ]
            o_sb = opool.tile([P, MO, FD], mybir.dt.float32)
            for ko in range(KO):
                for mo in range(MO):
                    nc.tensor.matmul(
                        ps0[mo][:],
                        w_sb[:, ko, mo],
                        x_sb[:, 0, ko],
                        start=(ko == 0),
                        stop=(ko == KO - 1),
                    )
            for mo in range(MO):
                nc.scalar.activation(
                    o_sb[:, mo],
                    ps0[mo][:],
                    mybir.ActivationFunctionType.Identity,
                    bias=b_sb[:, mo : mo + 1],
                )
            nc.sync.dma_start(out[:, 0], o_sb[:])

            # bq1..3: output-tile-outer, one PSUM bank at a time.
            for bq in range(1, NB):
                o_sb = opool.tile([P, MO, FD], mybir.dt.float32)
                for mo in range(MO):
                    ps = ppool.tile([P, FD], mybir.dt.float32, tag="ps")
                    for ko in range(KO):
                        nc.tensor.matmul(
                            ps[:],
                            w_sb[:, ko, mo],
                            x_sb[:, bq, ko],
                            start=(ko == 0),
                            stop=(ko == KO - 1),
                        )
                    nc.scalar.activation(
                        o_sb[:, mo],
                        ps[:],
                        mybir.ActivationFunctionType.Identity,
                        bias=b_sb[:, mo : mo + 1],
                    )
                if bq < NB - 1:
                    nc.sync.dma_start(out[:, bq], o_sb[:])
                else:
                    # Finer pushes on the last chunk so the final store
                    # doesn't add a 1 MiB DMA to the kernel tail.
                    for mh in range(0, MO, 2):
                        nc.sync.dma_start(
                            out[:, bq, mh : mh + 2], o_sb[:, mh : mh + 2]
                        )

    nc.compile()
    return nc


def kernel(x, weight, bias):
    global LAST_RESULT
    from concourse.bass_utils import run_bass_kernel_spmd

    if "nc" not in _cache:
        _cache["nc"] = _build()
    nc = _cache["nc"]

    bf16 = ml_dtypes.bfloat16
    # w3[p, ko, mo, c] = W[mo*P + c, ko*P + p]
    wb = weight.astype(bf16).reshape(MO, P, KO, P)  # [mo, c, ko, p]
    w3 = np.ascontiguousarray(wb.transpose(3, 2, 0, 1))  # [p, ko, mo, c]
    # bias laid out [P, MO]: b[p, mo] = bias[mo*P + p]
    b_t = np.ascontiguousarray(bias.astype(np.float32).reshape(MO, P).T)

    in_maps = []
    for c in range(NCORES):
        xs = x[c * BS : (c + 1) * BS].astype(bf16)
        # xT4[p, bq, ko, fd] = x[bq*FD + fd, ko*P + p]
        xr = xs.reshape(NB, FD, KO, P)  # [bq, fd, ko, p]
        xT = np.ascontiguousarray(xr.transpose(3, 0, 2, 1))  # [p, bq, ko, fd]
        in_maps.append({"xT": xT, "w3": w3, "bias": b_t})

    res = run_bass_kernel_spmd(nc, in_maps, list(range(NCORES)))
    LAST_RESULT = res

    y = np.empty((BATCH, OUT_F), dtype=np.float32)
    for c in range(NCORES):
        o = res.results[c]["out"]  # [p, bq, mo, fd]
        y[c * BS : (c + 1) * BS] = o.transpose(1, 3, 2, 0).reshape(BS, OUT_F)
    return y


# revision 17
# speedup vs baseline: 1.0002x; 1.0002x over previous
"""Trainium2 Bass kernel for y = x @ W.T + b  (x: [16384,1024] f32,
W: [1024,1024] f32, b: [1024] f32) on 8 NeuronCores.

Data-parallel: x is split along batch into 8 shards of 2048 rows;
W and b are replicated. Each core computes its y shard with bf16
matmuls accumulating in fp32 PSUM; bias is fused into the PSUM->SBUF
eviction on the Scalar engine. Host-side we pre-transpose x (and W) to
put the contraction dim on SBUF partitions, so no on-chip transposes
are needed, and group DRAM layouts so every DMA is 128 long contiguous
runs (DMA issue time scales with descriptor rows).

Schedule per core (bq = one of 4 batch chunks of 512 rows):
- Dummy matmuls warm the PE clock gate (1.2 -> 2.4 GHz takes ~3.4 us
  of busy) while the first input DMAs are in flight.
- bq0 runs contraction-outer across all 8 PSUM banks, consuming one
  (w[ko], x[ko]) chunk pair per 1.7 us — matched to the observed DMA
  rate so the matmul stream never starves while inputs land.
- bq1..3 run output-tile-outer (one PSUM bank at a time), evicting
  each bank through the Scalar engine while later tiles compute.
- Outputs stream out per batch chunk; the last chunk is split into
  four stores so the final DMA doesn't lengthen the kernel tail.
"""

import sys

if "/opt/trn_rl_repo" not in sys.path:
    sys.path.insert(0, "/opt/trn_rl_repo")

import ml_dtypes
import numpy as np

# concourse's trace path imports antenv.axon_hooks, which this image lacks.
# Register a working NTFF-profile hook (via the axon PJRT .so) so tracing
# works when requested, degrading to no-op if anything is missing.
try:
    import antenv.axon_hooks  # noqa: F401
except ImportError:
    import types as _types

    def _make_hook():
        try:
            from trn_agent_boot.trn_boot import _ntff_profile_via_ctypes

            return _ntff_profile_via_ctypes("/opt/axon/libaxon_pjrt.so")
        except Exception:
            return None

    _hooks = _types.ModuleType("antenv.axon_hooks")
    _hooks.get_axon_ntff_profile_hook = _make_hook
    _hooks.set_axon_ntff_profile_hook = lambda h: None
    sys.modules["antenv.axon_hooks"] = _hooks

BATCH = 16384
IN_F = 1024
OUT_F = 1024
NCORES = 8
P = 128
KO = IN_F // P  # 8 contraction tiles
MO = OUT_F // P  # 8 output-feature tiles
BS = BATCH // NCORES  # 2048 rows per core
FD = 512  # matmul moving free dim (one PSUM bank of fp32)
NB = BS // FD  # 4 batch chunks per core

_cache = {}
LAST_RESULT = None


def _build():
    import concourse.mybir as mybir
    import concourse.tile as tile
    from concourse import bacc

    nc = bacc.Bacc(None, target_bir_lowering=False)
    # xT4[p, bq, ko, fd] = x[bq*FD + fd, ko*P + p]
    xT = nc.declare_dram_parameter(
        "xT", [P, NB, KO, FD], mybir.dt.bfloat16, isOutput=False
    )
    # w3[p, ko, mo, c] = W[mo*P + c, ko*P + p]  (ko-major: bq0 consumes
    # weights one ko chunk at a time)
    w3 = nc.declare_dram_parameter(
        "w3", [P, KO, MO, P], mybir.dt.bfloat16, isOutput=False
    )
    bias = nc.declare_dram_parameter("bias", [P, MO], mybir.dt.float32, isOutput=False)
    # out4[p, bq, mo, fd] = y[bq*FD + fd, mo*P + p]
    out = nc.declare_dram_parameter(
        "out", [P, NB, MO, FD], mybir.dt.float32, isOutput=True
    )

    with tile.TileContext(nc) as tc:
        with (
            tc.tile_pool(name="const", bufs=1) as cpool,
            tc.tile_pool(name="outp", bufs=3) as opool,
            tc.tile_pool(name="psum", bufs=8, space="PSUM") as ppool,
        ):
            x_sb = cpool.tile([P, NB, KO, FD], mybir.dt.bfloat16)
            w_sb = cpool.tile([P, KO, MO, P], mybir.dt.bfloat16)
            b_sb = cpool.tile([P, MO], mybir.dt.float32)
            # PE HAM warm-up (shares the psum pool slots with the real
            # accumulation tiles; it finishes before they are needed).
            wu_sb = cpool.tile([P, 256], mybir.dt.bfloat16)
            nc.any.memset(wu_sb[:], 0.0)
            wu_ps = ppool.tile([P, FD], mybir.dt.float32, tag="ps")
            for _ in range(24):
                nc.tensor.matmul(
                    wu_ps[:, :256], wu_sb[:, :P], wu_sb[:], start=True, stop=True
                )
            # DMA issue order matches consumption order: (w, x) chunk pairs
            # for bq0 one ko at a time, then the remaining batch chunks as
            # one large contiguous DMA each.
            for ko in range(KO):
                nc.sync.dma_start(w_sb[:, ko], w3[:, ko])
                nc.sync.dma_start(x_sb[:, 0, ko], xT[:, 0, ko])
            nc.sync.dma_start(b_sb[:], bias[:])
            for bq in range(1, NB):
                nc.sync.dma_start(x_sb[:, bq], xT[:, bq])

            # bq0: contraction-outer over all 8 PSUM banks.
            ps0 = [
                ppool.tile([P, FD], mybir.dt.float32, tag="ps", name=f"ps0_{mo}")
                for mo in range(MO)
            ]
            o_sb = opool.tile([P, MO, FD], mybir.dt.float32)
            for ko in range(KO):
                for mo in range(MO):
                    nc.tensor.matmul(
                        ps0[mo][:],
                        w_sb[:, ko, mo],
                        x_sb[:, 0, ko],
                        start=(ko == 0),
                        stop=(ko == KO - 1),
                    )
            for mo in range(MO):
                nc.scalar.activation(
                    o_sb[:, mo],
                    ps0[mo][:],
                    mybir.ActivationFunctionType.Identity,
                    bias=b_sb[:, mo : mo + 1],
                )
            nc.sync.dma_start(out[:, 0], o_sb[:])

            # bq1..3: output-tile-outer, one PSUM bank at a time.
            for bq in range(1, NB):
                o_sb = opool.tile([P, MO, FD], mybir.dt.float32)
                for mo in range(MO):
                    ps = ppool.tile([P, FD], mybir.dt.float32, tag="ps")
                    for ko in range(KO):
                        nc.tensor.matmul(
                            ps[:],
                            w_sb[:, ko, mo],
                            x_sb[:, bq, ko],
                            start=(ko == 0),
                            stop=(ko == KO - 1),
                        )
                    nc.scalar.activation(
                        o_sb[:, mo],
                        ps[:],
                        mybir.ActivationFunctionType.Identity,
                        bias=b_sb[:, mo : mo + 1],
                    )
                if bq < NB - 1:
                    nc.sync.dma_start(out[:, bq], o_sb[:])
                else:
                    # Finer pushes on the last chunk so the final store
                    # doesn't add a 1 MiB DMA to the kernel tail.
                    for mh in range(0, MO, 2):
                        nc.sync.dma_start(
                            out[:, bq, mh : mh + 2], o_sb[:, mh : mh + 2]
                        )

    nc.compile()
    return nc


def kernel(x, weight, bias):
    global LAST_RESULT
    from concourse.bass_utils import run_bass_kernel_spmd

    if "nc" not in _cache:
        _cache["nc"] = _build()
    nc = _cache["nc"]

    x = np.asarray(x, dtype=np.float32)
    weight = np.asarray(weight, dtype=np.float32)
    bias = np.asarray(bias, dtype=np.float32)

    bf16 = ml_dtypes.bfloat16
    # w3[p, ko, mo, c] = W[mo*P + c, ko*P + p]
    wb = weight.astype(bf16).reshape(MO, P, KO, P)  # [mo, c, ko, p]
    w3 = np.ascontiguousarray(wb.transpose(3, 2, 0, 1))  # [p, ko, mo, c]
    # bias laid out [P, MO]: b[p, mo] = bias[mo*P + p]
    b_t = np.ascontiguousarray(bias.astype(np.float32).reshape(MO, P).T)

    in_maps = []
    for c in range(NCORES):
        xs = x[c * BS : (c + 1) * BS].astype(bf16)
        # xT4[p, bq, ko, fd] = x[bq*FD + fd, ko*P + p]
        xr = xs.reshape(NB, FD, KO, P)  # [bq, fd, ko, p]
        xT = np.ascontiguousarray(xr.transpose(3, 0, 2, 1))  # [p, bq, ko, fd]
        in_maps.append({"xT": xT, "w3": w3, "bias": b_t})

    res = run_bass_kernel_spmd(nc, in_maps, list(range(NCORES)))
    LAST_RESULT = res

    y = np.empty((BATCH, OUT_F), dtype=np.float32)
    for c in range(NCORES):
        o = res.results[c]["out"]  # [p, bq, mo, fd]
        y[c * BS : (c + 1) * BS] = o.transpose(1, 3, 2, 0).reshape(BS, OUT_F)
    return y


# revision 20
# speedup vs baseline: 1.0154x; 1.0151x over previous
"""Trainium2 Bass kernel for y = x @ W.T + b  (x: [16384,1024] f32,
W: [1024,1024] f32, b: [1024] f32) on 8 NeuronCores.

Data-parallel: x is split along batch into 8 shards of 2048 rows;
W and b are replicated. Each core computes its y shard with bf16
matmuls accumulating in fp32 PSUM; bias is fused into the PSUM->SBUF
eviction on the Scalar engine. Host-side we pre-transpose x (and W) to
put the contraction dim on SBUF partitions, so no on-chip transposes
are needed, and group DRAM layouts so every DMA is 128 long contiguous
runs (DMA issue time scales with descriptor rows).

Schedule per core (bq = one of 4 batch chunks of 512 rows):
- Dummy matmuls warm the PE clock gate (1.2 -> 2.4 GHz takes ~3.4 us
  of busy) while the first input DMAs are in flight.
- bq0 runs contraction-outer across all 8 PSUM banks, consuming one
  (w[ko], x[ko]) chunk pair per 1.7 us — matched to the observed DMA
  rate so the matmul stream never starves while inputs land.
- bq1..3 run output-tile-outer (one PSUM bank at a time), evicting
  each bank through the Scalar engine while later tiles compute.
- Outputs stream out per batch chunk; the last chunk is split into
  four stores so the final DMA doesn't lengthen the kernel tail.
"""

import sys

if "/opt/trn_rl_repo" not in sys.path:
    sys.path.insert(0, "/opt/trn_rl_repo")

import ml_dtypes
import numpy as np

# concourse's trace path imports antenv.axon_hooks, which this image lacks.
# Register a working NTFF-profile hook (via the axon PJRT .so) so tracing
# works when requested, degrading to no-op if anything is missing.
try:
    import antenv.axon_hooks  # noqa: F401
except ImportError:
    import types as _types

    def _make_hook():
        try:
            from trn_agent_boot.trn_boot import _ntff_profile_via_ctypes

            return _ntff_profile_via_ctypes("/opt/axon/libaxon_pjrt.so")
        except Exception:
            return None

    _hooks = _types.ModuleType("antenv.axon_hooks")
    _hooks.get_axon_ntff_profile_hook = _make_hook
    _hooks.set_axon_ntff_profile_hook = lambda h: None
    sys.modules["antenv.axon_hooks"] = _hooks

BATCH = 16384
IN_F = 1024
OUT_F = 1024
NCORES = 8
P = 128
KO = IN_F // P  # 8 contraction tiles
MO = OUT_F // P  # 8 output-feature tiles
BS = BATCH // NCORES  # 2048 rows per core
FD = 512  # matmul moving free dim (one PSUM bank of fp32)
NB = BS // FD  # 4 batch chunks per core

_cache = {}
LAST_RESULT = None


def _build():
    import concourse.mybir as mybir
    import concourse.tile as tile
    from concourse import bacc

    nc = bacc.Bacc(None, target_bir_lowering=False)
    # xT4[p, bq, ko, fd] = x[bq*FD + fd, ko*P + p]
    xT = nc.declare_dram_parameter(
        "xT", [P, NB, KO, FD], mybir.dt.bfloat16, isOutput=False
    )
    # w3[p, ko, mo, c] = W[mo*P + c, ko*P + p]  (ko-major: bq0 consumes
    # weights one ko chunk at a time)
    w3 = nc.declare_dram_parameter(
        "w3", [P, KO, MO, P], mybir.dt.bfloat16, isOutput=False
    )
    bias = nc.declare_dram_parameter("bias", [P, MO], mybir.dt.float32, isOutput=False)
    # out4[p, bq, mo, fd] = y[bq*FD + fd, mo*P + p]
    out = nc.declare_dram_parameter(
        "out", [P, NB, MO, FD], mybir.dt.float32, isOutput=True
    )

    with tile.TileContext(nc) as tc:
        with (
            tc.tile_pool(name="const", bufs=1) as cpool,
            tc.tile_pool(name="outp", bufs=3) as opool,
            tc.tile_pool(name="psum", bufs=8, space="PSUM") as ppool,
        ):
            x_sb = cpool.tile([P, NB, KO, FD], mybir.dt.bfloat16)
            w_sb = cpool.tile([P, KO, MO, P], mybir.dt.bfloat16)
            b_sb = cpool.tile([P, MO], mybir.dt.float32)
            # PE HAM warm-up (shares the psum pool slots with the real
            # accumulation tiles; it finishes before they are needed).
            wu_sb = cpool.tile([P, 256], mybir.dt.bfloat16)
            nc.any.memset(wu_sb[:], 0.0)
            wu_ps = ppool.tile([P, FD], mybir.dt.float32, tag="ps")
            for _ in range(24):
                nc.tensor.matmul(
                    wu_ps[:, :256], wu_sb[:, :P], wu_sb[:], start=True, stop=True
                )
            # DMA issue order matches consumption order: (w, x) chunk pairs
            # for bq0 one ko at a time, then the remaining batch chunks as
            # one large contiguous DMA each.
            for ko in range(KO):
                nc.sync.dma_start(w_sb[:, ko], w3[:, ko])
                nc.sync.dma_start(x_sb[:, 0, ko], xT[:, 0, ko])
            nc.sync.dma_start(b_sb[:], bias[:])
            for bq in range(1, NB):
                nc.sync.dma_start(x_sb[:, bq], xT[:, bq])

            # bq0: contraction-outer over all 8 PSUM banks.
            ps0 = [
                ppool.tile([P, FD], mybir.dt.float32, tag="ps", name=f"ps0_{mo}")
                for mo in range(MO)
            ]
            o_sb = opool.tile([P, MO, FD], mybir.dt.float32)
            for ko in range(KO):
                for mo in range(MO):
                    nc.tensor.matmul(
                        ps0[mo][:],
                        w_sb[:, ko, mo],
                        x_sb[:, 0, ko],
                        start=(ko == 0),
                        stop=(ko == KO - 1),
                    )
            for mo in range(MO):
                nc.scalar.activation(
                    o_sb[:, mo],
                    ps0[mo][:],
                    mybir.ActivationFunctionType.Identity,
                    bias=b_sb[:, mo : mo + 1],
                )
            nc.sync.dma_start(out[:, 0], o_sb[:])

            # bq1..3: output-tile-outer, one PSUM bank at a time.
            for bq in range(1, NB):
                o_sb = opool.tile([P, MO, FD], mybir.dt.float32)
                for mo in range(MO):
                    ps = ppool.tile([P, FD], mybir.dt.float32, tag="ps")
                    for ko in range(KO):
                        nc.tensor.matmul(
                            ps[:],
                            w_sb[:, ko, mo],
                            x_sb[:, bq, ko],
                            start=(ko == 0),
                            stop=(ko == KO - 1),
                        )
                    nc.scalar.activation(
                        o_sb[:, mo],
                        ps[:],
                        mybir.ActivationFunctionType.Identity,
                        bias=b_sb[:, mo : mo + 1],
                    )
                if bq < NB - 1:
                    nc.sync.dma_start(out[:, bq], o_sb[:])
                else:
                    # Finer pushes on the last chunk so the final store
                    # doesn't add a 1 MiB DMA to the kernel tail.
                    nc.sync.dma_start(out[:, bq, 0:4], o_sb[:, 0:4])
                    nc.sync.dma_start(out[:, bq, 4:6], o_sb[:, 4:6])
                    nc.sync.dma_start(out[:, bq, 6:7], o_sb[:, 6:7])
                    nc.sync.dma_start(out[:, bq, 7:8], o_sb[:, 7:8])

    nc.compile()
    return nc


def kernel(x, weight, bias):
    global LAST_RESULT
    from concourse.bass_utils import run_bass_kernel_spmd

    if "nc" not in _cache:
        _cache["nc"] = _build()
    nc = _cache["nc"]

    x = np.asarray(x, dtype=np.float32)
    weight = np.asarray(weight, dtype=np.float32)
    bias = np.asarray(bias, dtype=np.float32)

    bf16 = ml_dtypes.bfloat16
    # w3[p, ko, mo, c] = W[mo*P + c, ko*P + p]
    wb = weight.astype(bf16).reshape(MO, P, KO, P)  # [mo, c, ko, p]
    w3 = np.ascontiguousarray(wb.transpose(3, 2, 0, 1))  # [p, ko, mo, c]
    # bias laid out [P, MO]: b[p, mo] = bias[mo*P + p]
    b_t = np.ascontiguousarray(bias.astype(np.float32).reshape(MO, P).T)

    in_maps = []
    for c in range(NCORES):
        xs = x[c * BS : (c + 1) * BS].astype(bf16)
        # xT4[p, bq, ko, fd] = x[bq*FD + fd, ko*P + p]
        xr = xs.reshape(NB, FD, KO, P)  # [bq, fd, ko, p]
        xT = np.ascontiguousarray(xr.transpose(3, 0, 2, 1))  # [p, bq, ko, fd]
        in_maps.append({"xT": xT, "w3": w3, "bias": b_t})

    res = run_bass_kernel_spmd(nc, in_maps, list(range(NCORES)))
    LAST_RESULT = res

    y = np.empty((BATCH, OUT_F), dtype=np.float32)
    for c in range(NCORES):
        o = res.results[c]["out"]  # [p, bq, mo, fd]
        y[c * BS : (c + 1) * BS] = o.transpose(1, 3, 2, 0).reshape(BS, OUT_F)
    return y


# revision 26
# speedup vs baseline: 1.0182x; 1.0028x over previous
"""Trainium2 Bass kernel for y = x @ W.T + b  (x: [16384,1024] f32,
W: [1024,1024] f32, b: [1024] f32) on 8 NeuronCores.

Data-parallel: x is split along batch into 8 shards of 2048 rows;
W and b are replicated. Each core computes its y shard with bf16
matmuls accumulating in fp32 PSUM; bias is fused into the PSUM->SBUF
eviction on the Scalar engine. Host-side we pre-transpose x (and W) to
put the contraction dim on SBUF partitions, so no on-chip transposes
are needed, and group DRAM layouts so every DMA is 128 long contiguous
runs (DMA issue time scales with descriptor rows).

Schedule per core (bq = one of 4 batch chunks of 512 rows):
- Dummy matmuls warm the PE clock gate (1.2 -> 2.4 GHz takes ~3.4 us
  of busy) while the first input DMAs are in flight.
- bq0 runs contraction-outer across all 8 PSUM banks, consuming one
  (w[ko], x[ko]) chunk pair per 1.7 us — matched to the observed DMA
  rate so the matmul stream never starves while inputs land.
- bq1..3 run output-tile-outer (one PSUM bank at a time), evicting
  each bank through the Scalar engine while later tiles compute.
- Outputs stream out per batch chunk; the last chunk is split into
  four stores so the final DMA doesn't lengthen the kernel tail.
"""

import sys

if "/opt/trn_rl_repo" not in sys.path:
    sys.path.insert(0, "/opt/trn_rl_repo")

import ml_dtypes
import numpy as np

# concourse's trace path imports antenv.axon_hooks, which this image lacks.
# Register a working NTFF-profile hook (via the axon PJRT .so) so tracing
# works when requested, degrading to no-op if anything is missing.
try:
    import antenv.axon_hooks  # noqa: F401
except ImportError:
    import types as _types

    def _make_hook():
        try:
            from trn_agent_boot.trn_boot import _ntff_profile_via_ctypes

            return _ntff_profile_via_ctypes("/opt/axon/libaxon_pjrt.so")
        except Exception:
            return None

    _hooks = _types.ModuleType("antenv.axon_hooks")
    _hooks.get_axon_ntff_profile_hook = _make_hook
    _hooks.set_axon_ntff_profile_hook = lambda h: None
    sys.modules["antenv.axon_hooks"] = _hooks

BATCH = 16384
IN_F = 1024
OUT_F = 1024
NCORES = 8
P = 128
KO = IN_F // P  # 8 contraction tiles
MO = OUT_F // P  # 8 output-feature tiles
BS = BATCH // NCORES  # 2048 rows per core
FD = 512  # matmul moving free dim (one PSUM bank of fp32)
NB = BS // FD  # 4 batch chunks per core

_cache = {}
LAST_RESULT = None


def _build():
    import concourse.mybir as mybir
    import concourse.tile as tile
    from concourse import bacc

    nc = bacc.Bacc(None, target_bir_lowering=False)
    # xT4[p, bq, ko, fd] = x[bq*FD + fd, ko*P + p]
    xT = nc.declare_dram_parameter(
        "xT", [P, NB, KO, FD], mybir.dt.bfloat16, isOutput=False
    )
    # w3[p, ko, mo, c] = W[mo*P + c, ko*P + p]  (ko-major: bq0 consumes
    # weights one ko chunk at a time)
    w3 = nc.declare_dram_parameter(
        "w3", [P, KO, MO, P], mybir.dt.bfloat16, isOutput=False
    )
    bias = nc.declare_dram_parameter("bias", [P, MO], mybir.dt.float32, isOutput=False)
    # out4[p, bq, mo, fd] = y[bq*FD + fd, mo*P + p]
    out = nc.declare_dram_parameter(
        "out", [P, NB, MO, FD], mybir.dt.float32, isOutput=True
    )

    with tile.TileContext(nc) as tc:
        with (
            tc.tile_pool(name="const", bufs=1) as cpool,
            tc.tile_pool(name="outp", bufs=3) as opool,
            tc.tile_pool(name="psum", bufs=8, space="PSUM") as ppool,
        ):
            x_sb = cpool.tile([P, NB, KO, FD], mybir.dt.bfloat16)
            w_sb = cpool.tile([P, KO, MO, P], mybir.dt.bfloat16)
            b_sb = cpool.tile([P, MO], mybir.dt.float32)
            # PE HAM warm-up (shares the psum pool slots with the real
            # accumulation tiles; it finishes before they are needed).
            wu_sb = cpool.tile([P, 256], mybir.dt.bfloat16)
            nc.any.memset(wu_sb[:], 0.0)
            wu_ps = ppool.tile([P, FD], mybir.dt.float32, tag="ps")
            for _ in range(24):
                nc.tensor.matmul(
                    wu_ps[:, :256], wu_sb[:, :P], wu_sb[:], start=True, stop=True
                )
            # DMA issue order matches consumption order: (w, x) chunk pairs
            # for bq0 one ko at a time, then the remaining batch chunks as
            # one large contiguous DMA each.
            for ko in range(KO):
                nc.sync.dma_start(w_sb[:, ko], w3[:, ko])
                nc.sync.dma_start(x_sb[:, 0, ko], xT[:, 0, ko])
            nc.sync.dma_start(b_sb[:], bias[:])
            for bq in range(1, NB):
                nc.sync.dma_start(x_sb[:, bq], xT[:, bq])

            # bq0: contraction-outer over all 8 PSUM banks.
            ps0 = [
                ppool.tile([P, FD], mybir.dt.float32, tag="ps", name=f"ps0_{mo}")
                for mo in range(MO)
            ]
            o_sb = opool.tile([P, MO, FD], mybir.dt.float32)
            for ko in range(KO):
                for mo in range(MO):
                    nc.tensor.matmul(
                        ps0[mo][:],
                        w_sb[:, ko, mo],
                        x_sb[:, 0, ko],
                        start=(ko == 0),
                        stop=(ko == KO - 1),
                    )
            # Alternate eviction engines so the 8 banks free ~2x faster —
            # bq1's first group is waiting on a slot at this point.
            for mo in range(MO):
                if mo % 2 == 0:
                    nc.scalar.activation(
                        o_sb[:, mo],
                        ps0[mo][:],
                        mybir.ActivationFunctionType.Identity,
                        bias=b_sb[:, mo : mo + 1],
                    )
                else:
                    nc.vector.tensor_scalar_add(
                        o_sb[:, mo], ps0[mo][:], b_sb[:, mo : mo + 1]
                    )
            nc.sync.dma_start(out[:, 0], o_sb[:])

            # bq1..3: output-tile-outer, one PSUM bank at a time.
            for bq in range(1, NB):
                o_sb = opool.tile([P, MO, FD], mybir.dt.float32)
                for mo in range(MO):
                    if bq == NB - 1 and mo == MO - 1:
                        # Very last tile: two half-width PSUM groups so the
                        # final evict+store chain handles 64 KiB, not 256.
                        for h in range(2):
                            hs = slice(h * (FD // 2), (h + 1) * (FD // 2))
                            ps = ppool.tile(
                                [P, FD], mybir.dt.float32, tag="ps", name=f"ps_l{h}"
                            )
                            for ko in range(KO):
                                nc.tensor.matmul(
                                    ps[:, : FD // 2],
                                    w_sb[:, ko, mo],
                                    x_sb[:, bq, ko, hs],
                                    start=(ko == 0),
                                    stop=(ko == KO - 1),
                                )
                            nc.scalar.activation(
                                o_sb[:, mo, hs],
                                ps[:, : FD // 2],
                                mybir.ActivationFunctionType.Identity,
                                bias=b_sb[:, mo : mo + 1],
                            )
                            nc.sync.dma_start(out[:, bq, mo, hs], o_sb[:, mo, hs])
                    else:
                        ps = ppool.tile([P, FD], mybir.dt.float32, tag="ps")
                        for ko in range(KO):
                            nc.tensor.matmul(
                                ps[:],
                                w_sb[:, ko, mo],
                                x_sb[:, bq, ko],
                                start=(ko == 0),
                                stop=(ko == KO - 1),
                            )
                        nc.scalar.activation(
                            o_sb[:, mo],
                            ps[:],
                            mybir.ActivationFunctionType.Identity,
                            bias=b_sb[:, mo : mo + 1],
                        )
                if bq < NB - 1:
                    nc.sync.dma_start(out[:, bq], o_sb[:])
                else:
                    # Finer pushes on the last chunk so the final store
                    # doesn't add a 1 MiB DMA to the kernel tail.
                    nc.sync.dma_start(out[:, bq, 0:4], o_sb[:, 0:4])
                    nc.sync.dma_start(out[:, bq, 4:6], o_sb[:, 4:6])
                    nc.sync.dma_start(out[:, bq, 6:7], o_sb[:, 6:7])

    nc.compile()
    return nc


def kernel(x, weight, bias):
    global LAST_RESULT
    from concourse.bass_utils import run_bass_kernel_spmd

    if "nc" not in _cache:
        _cache["nc"] = _build()
    nc = _cache["nc"]

    x = np.asarray(x, dtype=np.float32)
    weight = np.asarray(weight, dtype=np.float32)
    bias = np.asarray(bias, dtype=np.float32)

    bf16 = ml_dtypes.bfloat16
    # w3[p, ko, mo, c] = W[mo*P + c, ko*P + p]
    wb = weight.astype(bf16).reshape(MO, P, KO, P)  # [mo, c, ko, p]
    w3 = np.ascontiguousarray(wb.transpose(3, 2, 0, 1))  # [p, ko, mo, c]
    # bias laid out [P, MO]: b[p, mo] = bias[mo*P + p]
    b_t = np.ascontiguousarray(bias.astype(np.float32).reshape(MO, P).T)

    in_maps = []
    for c in range(NCORES):
        xs = x[c * BS : (c + 1) * BS].astype(bf16)
        # xT4[p, bq, ko, fd] = x[bq*FD + fd, ko*P + p]
        xr = xs.reshape(NB, FD, KO, P)  # [bq, fd, ko, p]
        xT = np.ascontiguousarray(xr.transpose(3, 0, 2, 1))  # [p, bq, ko, fd]
        in_maps.append({"xT": xT, "w3": w3, "bias": b_t})

    res = run_bass_kernel_spmd(nc, in_maps, list(range(NCORES)))
    LAST_RESULT = res

    y = np.empty((BATCH, OUT_F), dtype=np.float32)
    for c in range(NCORES):
        o = res.results[c]["out"]  # [p, bq, mo, fd]
        y[c * BS : (c + 1) * BS] = o.transpose(1, 3, 2, 0).reshape(BS, OUT_F)
    return y
